# revision 22
# baseline (speedup 1.0000x reference)
"""Multi-head attention (B=2, S=2048, D=1024, 16 heads x 64) on 8 NeuronCores.

Sharding: batch x head-group data/tensor parallel. Core c handles batch
c//4 and heads [4*(c%4), 4*(c%4)+4). Wq/Wk/Wv are column-sliced per head
group, Wo row-sliced; each core emits a partial [S, D] output and the host
sums the 4 partials per batch (row-parallel reduce done host-side) and
adds bo.

Matmul data is fp16: 1 cycle/row on the PE at any N (fp32r measured 3x
slower: walrus lowers it to HIGH/LOW two-pass matmuls), fast weight loads,
half the DMA bytes. The data ranges here (inputs ~N(0,1), weights ~0.02,
scores ~N(0,0.17), exp <= ~13) sit comfortably inside fp16's range, and
fp16's 10-bit mantissa keeps per-matmul quantization error ~6e-4.
Accumulation is always fp32 in PSUM.

Per-core kernel:
  1. Q^T, K^T, V^T = (W.T @ X.T) projections in transposed layout
     [inner, seq]; per-partition bias add on the PSUM->SBUF move.
  2. V^T is PE-transposed to natural [seq, dh] chunks stored with a ones
     column appended (V|1) so attention row-sums fall out of the attn@V
     matmul.
  3. Per (head, q-slab of 1024): scoresT[k,q] = K^T_tile.T @ Q^T (16
     k-chunk matmuls), ACT exp (scale=1/8 folded in; no max-subtraction:
     scores are O(1) by construction), attnV accumulation
     [V|1].T @ expT -> [65, 1024] PSUM with row 64 = softmax denominator.
     Normalize: recip_approx_fast + K=1 PE outer-product broadcast + DVE
     multiply into O^T.
  4. Output projection: natural-layout final tiles = O^T_chunk.T @ Wo.
"""

import sys

if "/opt/trn_rl_repo" not in sys.path:
    sys.path.insert(0, "/opt/trn_rl_repo")

import numpy as np

import concourse.bacc as bacc
import concourse.mybir as mybir
import concourse.tile as tile
from concourse.bass_utils import run_bass_kernel_spmd
from concourse.masks import make_identity

F32 = mybir.dt.float32
F16 = mybir.dt.float16
NPDT = np.float16

B, S, D = 2, 2048, 1024
NH, DH = 16, 64
NCORES = 8
GROUPS = 4                # head-groups (cores per batch)
HG = NH // GROUPS         # heads per core = 4
IS = HG * DH              # inner slice per core = 256
KD = D // 128             # contraction chunks for projections = 8
MT = IS // 128            # m-tiles of the inner slice = 2
W = 1024                  # seq slab width (fp16 moving-operand max)
NSLAB = S // W            # seq slabs = 2
KT = S // 128             # 128-row key chunks = 16
QT = S // 128             # 128-row query tiles = 16

_CACHE = {}


def _build_nc():
    nc = bacc.Bacc("TRN2", target_bir_lowering=False, debug=False)

    xqT = nc.dram_tensor("xqT", [D, S], F16, kind="ExternalInput").ap()
    xkT = nc.dram_tensor("xkT", [D, S], F16, kind="ExternalInput").ap()
    xvT = nc.dram_tensor("xvT", [D, S], F16, kind="ExternalInput").ap()
    wq = nc.dram_tensor("wq", [D, IS], F16, kind="ExternalInput").ap()
    wk = nc.dram_tensor("wk", [D, IS], F16, kind="ExternalInput").ap()
    wv = nc.dram_tensor("wv", [D, IS], F16, kind="ExternalInput").ap()
    wo = nc.dram_tensor("wo", [IS, D], F16, kind="ExternalInput").ap()
    bq = nc.dram_tensor("bq", [IS], F32, kind="ExternalInput").ap()
    bk = nc.dram_tensor("bk", [IS], F32, kind="ExternalInput").ap()
    bv = nc.dram_tensor("bv", [IS], F32, kind="ExternalInput").ap()
    out = nc.dram_tensor("out", [S, D], F32, kind="ExternalOutput").ap()

    with tile.TileContext(nc) as tc:
        _emit(nc, tc, xqT, xkT, xvT, wq, wk, wv, wo, bq, bk, bv, out)
    nc.compile()
    return nc


def _emit(nc, tc, xqT, xkT, xvT, wq, wk, wv, wo, bq, bk, bv, out):
    from contextlib import ExitStack

    def mm(out_ap, lhsT, rhs, start, stop):
        """Matmul with the fp16 moving-operand N<=512 limit: slice the
        free dim of out/rhs into 512-wide chunks."""
        n = rhs.shape[-1]
        assert out_ap.shape[-1] == n
        for o in range(0, n, 512):
            w = min(512, n - o)
            nc.tensor.matmul(
                out_ap[..., o:o + w], lhsT, rhs[..., o:o + w],
                start=start, stop=stop,
            )

    ctx = ExitStack()
    with ctx:
        consts = ctx.enter_context(tc.tile_pool(name="consts", bufs=1))
        big = ctx.enter_context(tc.tile_pool(name="big", bufs=1))
        xin = ctx.enter_context(tc.tile_pool(name="xin", bufs=6))
        expp = ctx.enter_context(tc.tile_pool(name="expp", bufs=6))
        smallp = ctx.enter_context(tc.tile_pool(name="smallp", bufs=2))
        outp = ctx.enter_context(tc.tile_pool(name="outp", bufs=3))
        pobp = ctx.enter_context(tc.tile_pool(name="pobp", bufs=16))

        # ---- constants (weight/bias DMAs are emitted per-stage below so
        # the first projection's inputs go out on the wire first) ----
        ident = consts.tile([128, 128], F16, name="ident")
        make_identity(nc, ident)
        col1_f = consts.tile([128, 1], F16, name="col1_f")
        nc.vector.memset(col1_f, 1.0)
        wq_sb = consts.tile([128, KD, IS], F16, name="wq_sb")
        wk_sb = consts.tile([128, KD, IS], F16, name="wk_sb")
        wv_sb = consts.tile([128, KD, IS], F16, name="wv_sb")
        wo_sb = consts.tile([128, MT, D], F16, name="wo_sb")
        bq_sb = consts.tile([128, MT], F32, name="bq_sb")
        bk_sb = consts.tile([128, MT], F32, name="bk_sb")
        bv_sb = consts.tile([128, MT], F32, name="bv_sb")

        # ---- persistent intermediates ----
        # Q^T/K^T are stored per-head, zero-padded from dh=64 to a full
        # 128-row contraction, and V chunks are zero-padded from 65 to
        # 128 columns: a K=64 / M=65 matmul only lights half the PE
        # array's row/col groups and the HAM clock gate then never sees
        # "busy", locking the PE at 1.2 GHz through all of attention.
        # The padding costs nothing (matmul time is N cycles regardless
        # of K and M) and keeps the array at 2.4 GHz.
        QT_sb = big.tile([128, HG, S], F16, name="QT_sb")
        KT_sb = big.tile([128, HG, S], F16, name="KT_sb")
        VT_sb = big.tile([128, MT, S], F16, name="VT_sb")
        V_sb = big.tile([128, HG * KT, 128], F16, name="V_sb")
        OT_sb = big.tile([128, MT, S], F16, name="OT_sb")
        nc.gpsimd.memset(QT_sb, 0.0)
        nc.gpsimd.memset(KT_sb, 0.0)
        nc.gpsimd.memset(V_sb, 0.0)
        # the "ones" column (col DH) of every V chunk
        nc.vector.tensor_copy(
            V_sb[:, :, DH:DH + 1],
            col1_f.unsqueeze(1).broadcast_to([128, HG * KT, 1]),
        )

        # PSUM->SBUF moves with bias add, alternating between DVE and the
        # (otherwise idle in stage 1) ACT engine
        _eng = [0]

        def bias_copy(dst_ap, src_ap, bias_ap):
            if _eng[0] % 2 == 0:
                nc.vector.tensor_scalar_add(dst_ap, src_ap, bias_ap)
            else:
                nc.scalar.activation(
                    dst_ap, src_ap,
                    mybir.ActivationFunctionType.Identity,
                    bias=bias_ap,
                )
            _eng[0] += 1

        # ---- stage 1: projections into transposed layout [inner, seq] ----
        NS2 = S // 512
        with nc.named_scope("proj"):
            with tc.tile_pool(name="psP", bufs=8, space="PSUM") as psP:
                first = True
                for xT, w_dram, w_sb, b_dram, b_sb, dest in (
                    (xkT, wk, wk_sb, bk, bk_sb, KT_sb),
                    (xqT, wq, wq_sb, bq, bq_sb, QT_sb),
                    (xvT, wv, wv_sb, bv, bv_sb, VT_sb),
                ):
                    nc.sync.dma_start(
                        out=w_sb, in_=w_dram.rearrange("(k p) i -> p k i", p=128)
                    )
                    nc.sync.dma_start(
                        out=b_sb, in_=b_dram.rearrange("(m p) -> p m", p=128)
                    )
                    ps = [
                        [
                            psP.tile([128, 512], F32, tag="ps", name=f"ps_{m}_{n}")
                            for n in range(NS2)
                        ]
                        for m in range(MT)
                    ]
                    for k in range(KD):
                        xt = xin.tile([128, S], F16, tag="xt")
                        nc.sync.dma_start(out=xt, in_=xT[128 * k:128 * (k + 1), :])
                        if first and k == 0:
                            # HAM warm-up: burn ~7us of PE on a throwaway
                            # pass over chunk 0 while the remaining input
                            # DMAs stream in, so the clock gate opens
                            # before the real work.
                            for m in range(MT):
                                for n in range(NS2):
                                    nc.tensor.matmul(
                                        ps[m][n],
                                        w_sb[:, 0, 128 * m:128 * (m + 1)],
                                        xt[:, 512 * n:512 * (n + 1)],
                                        start=True, stop=True)
                            first = False
                        for m in range(MT):
                            for n in range(NS2):
                                nc.tensor.matmul(
                                    ps[m][n],
                                    w_sb[:, k, 128 * m:128 * (m + 1)],
                                    xt[:, 512 * n:512 * (n + 1)],
                                    start=(k == 0),
                                    stop=(k == KD - 1),
                                )
                    for m in range(MT):
                        for n in range(NS2):
                            if dest is VT_sb:
                                bias_copy(
                                    dest[:, m, 512 * n:512 * (n + 1)],
                                    ps[m][n],
                                    b_sb[:, m:m + 1],
                                )
                            else:
                                for hh in range(2):
                                    bias_copy(
                                        dest[0:DH, 2 * m + hh, 512 * n:512 * (n + 1)],
                                        ps[m][n][64 * hh:64 * hh + DH, :],
                                        b_sb[64 * hh:64 * hh + DH, m:m + 1],
                                    )
                nc.sync.dma_start(
                    out=wo_sb, in_=wo.rearrange("(c p) d -> p c d", p=128)
                )

        # ---- stages 1.5 + 2 + 3 share one PSUM pool: tag "sc" (3x
        # 2-bank slots, reused by V-transposes, score tiles, and
        # output-projection tiles) + tag "av" (2x 1-bank halves) ----
        with tc.tile_pool(name="psX", bufs=2, space="PSUM") as psX:
            # ---- stage 1.5: V^T -> V natural chunks (PE transpose) ----
            with nc.named_scope("vtr"):
                for m in range(MT):
                    for j in range(KT):
                        trp = psX.tile([128, 128], F16, tag="sc", name="trp",
                                       bufs=3)
                        nc.tensor.transpose(
                            trp, VT_sb[:, m, 128 * j:128 * (j + 1)], ident
                        )
                        for hh in range(2):
                            h = 2 * m + hh
                            nc.vector.tensor_copy(
                                V_sb[:, h * KT + j, 0:DH],
                                trp[:, 64 * hh:64 * hh + DH],
                            )

            # ---- stages 2 + 3: one flat software-pipelined stream over
            # (head, k-chunk): attnV for chunk i-2 is emitted after
            # scores+exp for chunk i (the PE then never waits on the ACT
            # exp, which is the phase bottleneck, and a stalled PE
            # sequencer would re-throttle the clock), crossing head
            # boundaries so the ACT pipe never drains. Output-projection
            # tiles for the previous q-slab drip in as PE filler. ----
            obs = {}
            _foeng = [0]

            def emit_fo(t, half, tag="sc"):
                fp = psX.tile([128, 512], F32, tag=tag, name="fp",
                              bufs=3 if tag == "sc" else 2)
                for c in range(MT):
                    nc.tensor.matmul(
                        fp,
                        OT_sb[:, c, 128 * t:128 * (t + 1)],
                        wo_sb[:, c, 512 * half:512 * (half + 1)],
                        start=(c == 0),
                        stop=(c == MT - 1),
                    )
                if t not in obs:
                    obs[t] = outp.tile([128, D], F32, tag="ob", name="ob")
                dst = obs[t][:, 512 * half:512 * (half + 1)]
                if _foeng[0] % 2 == 0:
                    nc.vector.tensor_copy(dst, fp)
                else:
                    nc.scalar.activation(
                        dst, fp, mybir.ActivationFunctionType.Copy
                    )
                _foeng[0] += 1
                if half == 1:
                    nc.sync.dma_start(
                        out=out[128 * t:128 * (t + 1), :], in_=obs.pop(t)
                    )

            fo_queue = []
            po_queue = []
            pobs = {}

            def emit_po(t, half):
                # first half of the output projection (heads 0/1 chunk)
                # for the LAST slab, run as PE filler while heads 2/3 are
                # still in attention; the drain then only needs the c=1
                # matmul plus an add.
                fp = psX.tile([128, 512], F32, tag="sc", name="fp", bufs=3)
                nc.tensor.matmul(
                    fp,
                    OT_sb[:, 0, 128 * t:128 * (t + 1)],
                    wo_sb[:, 0, 512 * half:512 * (half + 1)],
                    start=True,
                    stop=True,
                )
                pob = pobp.tile([128, 512], F16, name="pob", tag="pob")
                if _foeng[0] % 2 == 0:
                    nc.vector.tensor_copy(pob, fp)
                else:
                    nc.scalar.activation(
                        pob, fp, mybir.ActivationFunctionType.Copy
                    )
                _foeng[0] += 1
                pobs[(t, half)] = pob

            def emit_fo2(t, half, tag):
                fp = psX.tile([128, 512], F32, tag=tag, name="fp",
                              bufs=3 if tag == "sc" else 2)
                nc.tensor.matmul(
                    fp,
                    OT_sb[:, 1, 128 * t:128 * (t + 1)],
                    wo_sb[:, 1, 512 * half:512 * (half + 1)],
                    start=True,
                    stop=True,
                )
                if t not in obs:
                    obs[t] = outp.tile([128, D], F32, tag="ob", name="ob")
                dst = obs[t][:, 512 * half:512 * (half + 1)]
                nc.vector.tensor_add(dst, fp, pobs.pop((t, half)))
                if half == 1:
                    nc.sync.dma_start(
                        out=out[128 * t:128 * (t + 1), :], in_=obs.pop(t)
                    )

            with nc.named_scope("attn"):
                avs = {}
                exs = {}

                def attnv(s, h, j):
                    if j == 0:
                        avs[(s, h)] = [
                            psX.tile([128, 512], F32, tag="av",
                                     name=f"av{half}", bufs=2)
                            for half in range(2)
                        ]
                    ex = exs.pop((s, h, j))
                    for half in range(2):
                        nc.tensor.matmul(
                            avs[(s, h)][half],
                            V_sb[:, h * KT + j, :],
                            ex[:, 512 * half:512 * (half + 1)],
                            start=(j == 0),
                            stop=(j == KT - 1),
                        )

                def norm(s, h):
                    m_h, p0 = h // 2, 64 * (h % 2)
                    av = avs.pop((s, h))
                    sums = smallp.tile([1, W], F32, tag="sums")
                    for half in range(2):
                        nc.vector.tensor_copy(
                            sums[:, 512 * half:512 * (half + 1)],
                            av[half][DH:DH + 1, :],
                        )
                    rec = smallp.tile([1, W], F32, tag="rec")
                    nc.vector.reciprocal_approx_fast(rec, sums)
                    bcs = smallp.tile([DH, W], F32, tag="bcs")
                    nc.gpsimd.partition_broadcast(bcs, rec)
                    for half in range(2):
                        nc.vector.tensor_mul(
                            OT_sb[p0:p0 + DH, m_h,
                                  W * s + 512 * half:W * s + 512 * (half + 1)],
                            av[half][0:DH, :],
                            bcs[:, 512 * half:512 * (half + 1)],
                        )
                    if h == HG - 1 and s < NSLAB - 1:
                        # slab finished: queue its output projection as
                        # PE filler for the rest of the stream
                        fo_queue.extend(
                            (t, half)
                            for t in range(W * s // 128, W * (s + 1) // 128)
                            for half in range(2)
                        )
                    if h == 1 and s == NSLAB - 1:
                        # heads 0/1 of the last slab done: their Wo chunk
                        # can project now
                        po_queue.extend(
                            (t, half)
                            for t in range(W * s // 128, W * (s + 1) // 128)
                            for half in range(2)
                        )

                NIT = NSLAB * HG * KT
                for idx in range(NIT + 2):
                    if idx < NIT:
                        s, hj = divmod(idx, HG * KT)
                        h, j = divmod(hj, KT)
                        sc = psX.tile([128, W], F32, tag="sc", name="sc",
                                      bufs=3)
                        mm(
                            sc,
                            KT_sb[:, h, 128 * j:128 * (j + 1)],
                            QT_sb[:, h, W * s:W * (s + 1)],
                            start=True,
                            stop=True,
                        )
                        ex = expp.tile([128, W], F16, tag="ex")
                        nc.scalar.activation(
                            ex, sc, mybir.ActivationFunctionType.Exp,
                            scale=0.125,
                        )
                        exs[(s, h, j)] = ex
                    if idx >= 2:
                        sl, hjl = divmod(idx - 2, HG * KT)
                        hl, jl = divmod(hjl, KT)
                        attnv(sl, hl, jl)
                        if jl == KT - 1:
                            norm(sl, hl)
                    if idx % 2 == 1:
                        if fo_queue:
                            emit_fo(*fo_queue.pop(0))
                        elif po_queue:
                            emit_po(*po_queue.pop(0))
            # drain the last slab's output projection (c=1 chunk + add
            # of the stored c=0 partial), alternating PSUM tags
            with nc.named_scope("outproj"):
                items = [
                    (t, half)
                    for t in range(W * (NSLAB - 1) // 128, S // 128)
                    for half in range(2)
                ]
                for i, (t, half) in enumerate(items):
                    emit_fo2(t, half, tag=("sc", "av")[i % 2])


def _get_nc():
    if "nc" not in _CACHE:
        _CACHE["nc"] = _build_nc()
    return _CACHE["nc"]


def make_in_maps(query, key, value, Wq, bq, Wk, bk, Wv, bv, Wo, bo):
    f32 = lambda a: np.asarray(a, dtype=np.float32)
    f16 = lambda a: np.ascontiguousarray(np.asarray(a, dtype=np.float32).astype(NPDT))
    query, key, value = f32(query), f32(key), f32(value)
    bq, bk, bv = (
        np.ascontiguousarray(f32(bq)),
        np.ascontiguousarray(f32(bk)),
        np.ascontiguousarray(f32(bv)),
    )
    Wq, Wk, Wv, Wo = f32(Wq), f32(Wk), f32(Wv), f32(Wo)

    xT = [[f16(x[b].T) for b in range(B)] for x in (query, key, value)]
    in_maps = []
    for c in range(NCORES):
        b, g = c // GROUPS, c % GROUPS
        sl = slice(IS * g, IS * (g + 1))
        in_maps.append({
            "xqT": xT[0][b],
            "xkT": xT[1][b],
            "xvT": xT[2][b],
            "wq": f16(Wq[:, sl]),
            "wk": f16(Wk[:, sl]),
            "wv": f16(Wv[:, sl]),
            "wo": f16(Wo[sl, :]),
            "bq": np.ascontiguousarray(bq[sl]),
            "bk": np.ascontiguousarray(bk[sl]),
            "bv": np.ascontiguousarray(bv[sl]),
        })
    return in_maps


def combine_outputs(results, bo):
    bo = np.asarray(bo, dtype=np.float32)
    out = np.empty((B, S, D), dtype=np.float32)
    for b in range(B):
        acc = results[b * GROUPS]["out"].astype(np.float32)
        for g in range(1, GROUPS):
            acc = acc + results[b * GROUPS + g]["out"]
        out[b] = acc + bo
    return out


def kernel(query, key, value, Wq, bq, Wk, bk, Wv, bv, Wo, bo):
    nc = _get_nc()
    in_maps = make_in_maps(query, key, value, Wq, bq, Wk, bk, Wv, bv, Wo, bo)
    try:
        res = run_bass_kernel_spmd(nc, in_maps, list(range(NCORES)))
    except Exception:
        # a fresh NEFF's first execution occasionally reports
        # NRT_EXEC_UNIT_UNRECOVERABLE; a retry reliably succeeds
        res = run_bass_kernel_spmd(nc, in_maps, list(range(NCORES)))
    return combine_outputs(res.results, bo)


# revision 23
# speedup vs baseline: 1.0398x; 1.0398x over previous
"""Multi-head attention (B=2, S=2048, D=1024, 16 heads x 64) on 8 NeuronCores.

Sharding: batch x head-group data/tensor parallel. Core c handles batch
c//4 and heads [4*(c%4), 4*(c%4)+4). Wq/Wk/Wv are column-sliced per head
group, Wo row-sliced; each core emits a partial [S, D] output and the host
sums the 4 partials per batch (row-parallel reduce done host-side) and
adds bo.

Matmul data is fp16: 1 cycle/row on the PE at any N (fp32r measured 3x
slower: walrus lowers it to HIGH/LOW two-pass matmuls), fast weight loads,
half the DMA bytes. The data ranges here (inputs ~N(0,1), weights ~0.02,
scores ~N(0,0.17), exp <= ~13) sit comfortably inside fp16's range, and
fp16's 10-bit mantissa keeps per-matmul quantization error ~6e-4.
Accumulation is always fp32 in PSUM.

Per-core kernel:
  1. Q^T, K^T, V^T = (W.T @ X.T) projections in transposed layout
     [inner, seq]; per-partition bias add on the PSUM->SBUF move.
  2. V^T is PE-transposed to natural [seq, dh] chunks stored with a ones
     column appended (V|1) so attention row-sums fall out of the attn@V
     matmul.
  3. Per (head, q-slab of 1024): scoresT[k,q] = K^T_tile.T @ Q^T (16
     k-chunk matmuls), ACT exp (scale=1/8 folded in; no max-subtraction:
     scores are O(1) by construction), attnV accumulation
     [V|1].T @ expT -> [65, 1024] PSUM with row 64 = softmax denominator.
     Normalize: recip_approx_fast + K=1 PE outer-product broadcast + DVE
     multiply into O^T.
  4. Output projection: natural-layout final tiles = O^T_chunk.T @ Wo.
"""

import sys

if "/opt/trn_rl_repo" not in sys.path:
    sys.path.insert(0, "/opt/trn_rl_repo")

import numpy as np

import concourse.bacc as bacc
import concourse.mybir as mybir
import concourse.tile as tile
from concourse.bass_utils import run_bass_kernel_spmd
from concourse.masks import make_identity

F32 = mybir.dt.float32
F16 = mybir.dt.float16
NPDT = np.float16

B, S, D = 2, 2048, 1024
NH, DH = 16, 64
NCORES = 8
GROUPS = 4                # head-groups (cores per batch)
HG = NH // GROUPS         # heads per core = 4
IS = HG * DH              # inner slice per core = 256
KD = D // 128             # contraction chunks for projections = 8
MT = IS // 128            # m-tiles of the inner slice = 2
W = 1024                  # seq slab width (fp16 moving-operand max)
NSLAB = S // W            # seq slabs = 2
KT = S // 128             # 128-row key chunks = 16
QT = S // 128             # 128-row query tiles = 16

_CACHE = {}


def _build_nc():
    nc = bacc.Bacc("TRN2", target_bir_lowering=False, debug=False)

    xqT = nc.dram_tensor("xqT", [D, S], F16, kind="ExternalInput").ap()
    xkT = nc.dram_tensor("xkT", [D, S], F16, kind="ExternalInput").ap()
    xvT = nc.dram_tensor("xvT", [D, S], F16, kind="ExternalInput").ap()
    wq = nc.dram_tensor("wq", [D, IS], F16, kind="ExternalInput").ap()
    wk = nc.dram_tensor("wk", [D, IS], F16, kind="ExternalInput").ap()
    wv = nc.dram_tensor("wv", [D, IS], F16, kind="ExternalInput").ap()
    wo = nc.dram_tensor("wo", [IS, D], F16, kind="ExternalInput").ap()
    bq = nc.dram_tensor("bq", [IS], F32, kind="ExternalInput").ap()
    bk = nc.dram_tensor("bk", [IS], F32, kind="ExternalInput").ap()
    bv = nc.dram_tensor("bv", [IS], F32, kind="ExternalInput").ap()
    out = nc.dram_tensor("out", [S, D], F32, kind="ExternalOutput").ap()

    with tile.TileContext(nc) as tc:
        _emit(nc, tc, xqT, xkT, xvT, wq, wk, wv, wo, bq, bk, bv, out)
    nc.compile()
    return nc


def _emit(nc, tc, xqT, xkT, xvT, wq, wk, wv, wo, bq, bk, bv, out):
    from contextlib import ExitStack

    def mm(out_ap, lhsT, rhs, start, stop):
        """Matmul with the fp16 moving-operand N<=512 limit: slice the
        free dim of out/rhs into 512-wide chunks."""
        n = rhs.shape[-1]
        assert out_ap.shape[-1] == n
        for o in range(0, n, 512):
            w = min(512, n - o)
            nc.tensor.matmul(
                out_ap[..., o:o + w], lhsT, rhs[..., o:o + w],
                start=start, stop=stop,
            )

    ctx = ExitStack()
    with ctx:
        consts = ctx.enter_context(tc.tile_pool(name="consts", bufs=1))
        big = ctx.enter_context(tc.tile_pool(name="big", bufs=1))
        xin = ctx.enter_context(tc.tile_pool(name="xin", bufs=6))
        expp = ctx.enter_context(tc.tile_pool(name="expp", bufs=6))
        smallp = ctx.enter_context(tc.tile_pool(name="smallp", bufs=2))
        outp = ctx.enter_context(tc.tile_pool(name="outp", bufs=3))

        # ---- constants (weight/bias DMAs are emitted per-stage below so
        # the first projection's inputs go out on the wire first) ----
        ident = consts.tile([128, 128], F16, name="ident")
        make_identity(nc, ident)
        col1_f = consts.tile([128, 1], F16, name="col1_f")
        nc.vector.memset(col1_f, 1.0)
        wq_sb = consts.tile([128, KD, IS], F16, name="wq_sb")
        wk_sb = consts.tile([128, KD, IS], F16, name="wk_sb")
        wv_sb = consts.tile([128, KD, IS], F16, name="wv_sb")
        wo_sb = consts.tile([128, MT, D], F16, name="wo_sb")
        bq_sb = consts.tile([128, MT], F32, name="bq_sb")
        bk_sb = consts.tile([128, MT], F32, name="bk_sb")
        bv_sb = consts.tile([128, MT], F32, name="bv_sb")

        # ---- persistent intermediates ----
        # Q^T/K^T are stored per-head, zero-padded from dh=64 to a full
        # 128-row contraction, and V chunks are zero-padded from 65 to
        # 128 columns: a K=64 / M=65 matmul only lights half the PE
        # array's row/col groups and the HAM clock gate then never sees
        # "busy", locking the PE at 1.2 GHz through all of attention.
        # The padding costs nothing (matmul time is N cycles regardless
        # of K and M) and keeps the array at 2.4 GHz.
        QT_sb = big.tile([128, HG, S], F16, name="QT_sb")
        KT_sb = big.tile([128, HG, S], F16, name="KT_sb")
        VT_sb = big.tile([128, MT, S], F16, name="VT_sb")
        V_sb = big.tile([128, HG * KT, 128], F16, name="V_sb")
        OT_sb = big.tile([128, MT, S], F16, name="OT_sb")
        nc.gpsimd.memset(QT_sb, 0.0)
        nc.gpsimd.memset(KT_sb, 0.0)
        nc.gpsimd.memset(V_sb, 0.0)
        # the "ones" column (col DH) of every V chunk
        nc.vector.tensor_copy(
            V_sb[:, :, DH:DH + 1],
            col1_f.unsqueeze(1).broadcast_to([128, HG * KT, 1]),
        )

        # PSUM->SBUF moves with bias add, alternating between DVE and the
        # (otherwise idle in stage 1) ACT engine
        _eng = [0]

        def bias_copy(dst_ap, src_ap, bias_ap):
            if _eng[0] % 2 == 0:
                nc.vector.tensor_scalar_add(dst_ap, src_ap, bias_ap)
            else:
                nc.scalar.activation(
                    dst_ap, src_ap,
                    mybir.ActivationFunctionType.Identity,
                    bias=bias_ap,
                )
            _eng[0] += 1

        # ---- stage 1: projections into transposed layout [inner, seq] ----
        NS2 = S // 512
        with nc.named_scope("proj"):
            with tc.tile_pool(name="psP", bufs=8, space="PSUM") as psP:
                first = True
                for xT, w_dram, w_sb, b_dram, b_sb, dest in (
                    (xkT, wk, wk_sb, bk, bk_sb, KT_sb),
                    (xqT, wq, wq_sb, bq, bq_sb, QT_sb),
                    (xvT, wv, wv_sb, bv, bv_sb, VT_sb),
                ):
                    nc.sync.dma_start(
                        out=w_sb, in_=w_dram.rearrange("(k p) i -> p k i", p=128)
                    )
                    nc.sync.dma_start(
                        out=b_sb, in_=b_dram.rearrange("(m p) -> p m", p=128)
                    )
                    ps = [
                        [
                            psP.tile([128, 512], F32, tag="ps", name=f"ps_{m}_{n}")
                            for n in range(NS2)
                        ]
                        for m in range(MT)
                    ]
                    for k in range(KD):
                        xt = xin.tile([128, S], F16, tag="xt")
                        nc.sync.dma_start(out=xt, in_=xT[128 * k:128 * (k + 1), :])
                        if first and k == 0:
                            # HAM warm-up: burn ~7us of PE on a throwaway
                            # pass over chunk 0 while the remaining input
                            # DMAs stream in, so the clock gate opens
                            # before the real work.
                            for m in range(MT):
                                for n in range(NS2):
                                    nc.tensor.matmul(
                                        ps[m][n],
                                        w_sb[:, 0, 128 * m:128 * (m + 1)],
                                        xt[:, 512 * n:512 * (n + 1)],
                                        start=True, stop=True)
                            first = False
                        for m in range(MT):
                            for n in range(NS2):
                                nc.tensor.matmul(
                                    ps[m][n],
                                    w_sb[:, k, 128 * m:128 * (m + 1)],
                                    xt[:, 512 * n:512 * (n + 1)],
                                    start=(k == 0),
                                    stop=(k == KD - 1),
                                )
                    for m in range(MT):
                        for n in range(NS2):
                            if dest is VT_sb:
                                bias_copy(
                                    dest[:, m, 512 * n:512 * (n + 1)],
                                    ps[m][n],
                                    b_sb[:, m:m + 1],
                                )
                            else:
                                for hh in range(2):
                                    bias_copy(
                                        dest[0:DH, 2 * m + hh, 512 * n:512 * (n + 1)],
                                        ps[m][n][64 * hh:64 * hh + DH, :],
                                        b_sb[64 * hh:64 * hh + DH, m:m + 1],
                                    )
                nc.sync.dma_start(
                    out=wo_sb, in_=wo.rearrange("(c p) d -> p c d", p=128)
                )

        # ---- stages 1.5 + 2 + 3 share one PSUM pool: tag "sc" (3x
        # 2-bank slots, reused by V-transposes, score tiles, and
        # output-projection tiles) + tag "av" (2x 1-bank halves) ----
        with tc.tile_pool(name="psX", bufs=2, space="PSUM") as psX:
            # ---- stage 1.5: V^T -> V natural chunks (PE transpose) ----
            with nc.named_scope("vtr"):
                for m in range(MT):
                    for j in range(KT):
                        trp = psX.tile([128, 128], F16, tag="sc", name="trp",
                                       bufs=3)
                        nc.tensor.transpose(
                            trp, VT_sb[:, m, 128 * j:128 * (j + 1)], ident
                        )
                        for hh in range(2):
                            h = 2 * m + hh
                            nc.vector.tensor_copy(
                                V_sb[:, h * KT + j, 0:DH],
                                trp[:, 64 * hh:64 * hh + DH],
                            )

            # ---- stages 2 + 3: one flat software-pipelined stream over
            # (head, k-chunk): attnV for chunk i-2 is emitted after
            # scores+exp for chunk i (the PE then never waits on the ACT
            # exp, which is the phase bottleneck, and a stalled PE
            # sequencer would re-throttle the clock), crossing head
            # boundaries so the ACT pipe never drains. Output-projection
            # tiles for the previous q-slab drip in as PE filler. ----
            obs = {}
            _foeng = [0]

            def emit_fo(t, half, tag="sc"):
                fp = psX.tile([128, 512], F32, tag=tag, name="fp",
                              bufs=3 if tag == "sc" else 2)
                for c in range(MT):
                    nc.tensor.matmul(
                        fp,
                        OT_sb[:, c, 128 * t:128 * (t + 1)],
                        wo_sb[:, c, 512 * half:512 * (half + 1)],
                        start=(c == 0),
                        stop=(c == MT - 1),
                    )
                if t not in obs:
                    obs[t] = outp.tile([128, D], F32, tag="ob", name="ob")
                dst = obs[t][:, 512 * half:512 * (half + 1)]
                if _foeng[0] % 2 == 0:
                    nc.vector.tensor_copy(dst, fp)
                else:
                    nc.scalar.activation(
                        dst, fp, mybir.ActivationFunctionType.Copy
                    )
                _foeng[0] += 1
                if half == 1:
                    nc.sync.dma_start(
                        out=out[128 * t:128 * (t + 1), :], in_=obs.pop(t)
                    )

            fo_queue = []
            with nc.named_scope("attn"):
                avs = {}
                exs = {}

                def attnv(s, h, j):
                    if j == 0:
                        avs[(s, h)] = [
                            psX.tile([128, 512], F32, tag="av",
                                     name=f"av{half}", bufs=2)
                            for half in range(2)
                        ]
                    ex = exs.pop((s, h, j))
                    for half in range(2):
                        nc.tensor.matmul(
                            avs[(s, h)][half],
                            V_sb[:, h * KT + j, :],
                            ex[:, 512 * half:512 * (half + 1)],
                            start=(j == 0),
                            stop=(j == KT - 1),
                        )

                def norm(s, h):
                    m_h, p0 = h // 2, 64 * (h % 2)
                    av = avs.pop((s, h))
                    sums = smallp.tile([1, W], F32, tag="sums")
                    for half in range(2):
                        nc.vector.tensor_copy(
                            sums[:, 512 * half:512 * (half + 1)],
                            av[half][DH:DH + 1, :],
                        )
                    rec = smallp.tile([1, W], F32, tag="rec")
                    nc.vector.reciprocal_approx_fast(rec, sums)
                    bcs = smallp.tile([DH, W], F32, tag="bcs")
                    nc.gpsimd.partition_broadcast(bcs, rec)
                    for half in range(2):
                        nc.vector.tensor_mul(
                            OT_sb[p0:p0 + DH, m_h,
                                  W * s + 512 * half:W * s + 512 * (half + 1)],
                            av[half][0:DH, :],
                            bcs[:, 512 * half:512 * (half + 1)],
                        )
                    if h == HG - 1:
                        # slab finished: queue its output projection as
                        # PE filler for the rest of the stream
                        fo_queue.extend(
                            (t, half)
                            for t in range(W * s // 128, W * (s + 1) // 128)
                            for half in range(2)
                        )

                NIT = NSLAB * HG * KT
                for idx in range(NIT + 2):
                    if idx < NIT:
                        s, hj = divmod(idx, HG * KT)
                        h, j = divmod(hj, KT)
                        sc = psX.tile([128, W], F32, tag="sc", name="sc",
                                      bufs=3)
                        mm(
                            sc,
                            KT_sb[:, h, 128 * j:128 * (j + 1)],
                            QT_sb[:, h, W * s:W * (s + 1)],
                            start=True,
                            stop=True,
                        )
                        ex = expp.tile([128, W], F16, tag="ex")
                        nc.scalar.activation(
                            ex, sc, mybir.ActivationFunctionType.Exp,
                            scale=0.125,
                        )
                        exs[(s, h, j)] = ex
                    if idx >= 2:
                        sl, hjl = divmod(idx - 2, HG * KT)
                        hl, jl = divmod(hjl, KT)
                        attnv(sl, hl, jl)
                        if jl == KT - 1:
                            norm(sl, hl)
                    if idx % 4 == 3 and fo_queue:
                        emit_fo(*fo_queue.pop(0))
            # drain the last slab's output projection, alternating PSUM
            # tags for a deeper pipeline
            with nc.named_scope("outproj"):
                for i, item in enumerate(fo_queue):
                    emit_fo(*item, tag=("sc", "av")[i % 2])


def _get_nc():
    if "nc" not in _CACHE:
        _CACHE["nc"] = _build_nc()
    return _CACHE["nc"]


def make_in_maps(query, key, value, Wq, bq, Wk, bk, Wv, bv, Wo, bo):
    f32 = lambda a: np.asarray(a, dtype=np.float32)
    f16 = lambda a: np.ascontiguousarray(np.asarray(a, dtype=np.float32).astype(NPDT))
    query, key, value = f32(query), f32(key), f32(value)
    bq, bk, bv = (
        np.ascontiguousarray(f32(bq)),
        np.ascontiguousarray(f32(bk)),
        np.ascontiguousarray(f32(bv)),
    )
    Wq, Wk, Wv, Wo = f32(Wq), f32(Wk), f32(Wv), f32(Wo)

    xT = [[f16(x[b].T) for b in range(B)] for x in (query, key, value)]
    in_maps = []
    for c in range(NCORES):
        b, g = c // GROUPS, c % GROUPS
        sl = slice(IS * g, IS * (g + 1))
        in_maps.append({
            "xqT": xT[0][b],
            "xkT": xT[1][b],
            "xvT": xT[2][b],
            "wq": f16(Wq[:, sl]),
            "wk": f16(Wk[:, sl]),
            "wv": f16(Wv[:, sl]),
            "wo": f16(Wo[sl, :]),
            "bq": np.ascontiguousarray(bq[sl]),
            "bk": np.ascontiguousarray(bk[sl]),
            "bv": np.ascontiguousarray(bv[sl]),
        })
    return in_maps


def combine_outputs(results, bo):
    bo = np.asarray(bo, dtype=np.float32)
    out = np.empty((B, S, D), dtype=np.float32)
    for b in range(B):
        acc = results[b * GROUPS]["out"].astype(np.float32)
        for g in range(1, GROUPS):
            acc = acc + results[b * GROUPS + g]["out"]
        out[b] = acc + bo
    return out


def kernel(query, key, value, Wq, bq, Wk, bk, Wv, bv, Wo, bo):
    nc = _get_nc()
    in_maps = make_in_maps(query, key, value, Wq, bq, Wk, bk, Wv, bv, Wo, bo)
    try:
        res = run_bass_kernel_spmd(nc, in_maps, list(range(NCORES)))
    except Exception:
        # a fresh NEFF's first execution occasionally reports
        # NRT_EXEC_UNIT_UNRECOVERABLE; a retry reliably succeeds
        res = run_bass_kernel_spmd(nc, in_maps, list(range(NCORES)))
    return combine_outputs(res.results, bo)


# revision 25
# speedup vs baseline: 1.0817x; 1.0403x over previous
"""Multi-head attention (B=2, S=2048, D=1024, 16 heads x 64) on 8 NeuronCores.

Sharding: batch x head-group data/tensor parallel. Core c handles batch
c//4 and heads [4*(c%4), 4*(c%4)+4). Wq/Wk/Wv are column-sliced per head
group, Wo row-sliced; each core emits a partial [S, D] output and the host
sums the 4 partials per batch (row-parallel reduce done host-side) and
adds bo.

Matmul data is fp16: 1 cycle/row on the PE at any N (fp32r measured 3x
slower: walrus lowers it to HIGH/LOW two-pass matmuls), fast weight loads,
half the DMA bytes. The data ranges here (inputs ~N(0,1), weights ~0.02,
scores ~N(0,0.17), exp <= ~13) sit comfortably inside fp16's range, and
fp16's 10-bit mantissa keeps per-matmul quantization error ~6e-4.
Accumulation is always fp32 in PSUM.

Per-core kernel:
  1. Q^T, K^T, V^T = (W.T @ X.T) projections in transposed layout
     [inner, seq]; per-partition bias add on the PSUM->SBUF move.
  2. V^T is PE-transposed to natural [seq, dh] chunks stored with a ones
     column appended (V|1) so attention row-sums fall out of the attn@V
     matmul.
  3. Per (head, q-slab of 1024): scoresT[k,q] = K^T_tile.T @ Q^T (16
     k-chunk matmuls), ACT exp (scale=1/8 folded in; no max-subtraction:
     scores are O(1) by construction), attnV accumulation
     [V|1].T @ expT -> [65, 1024] PSUM with row 64 = softmax denominator.
     Normalize: recip_approx_fast + K=1 PE outer-product broadcast + DVE
     multiply into O^T.
  4. Output projection: natural-layout final tiles = O^T_chunk.T @ Wo.
"""

import sys

if "/opt/trn_rl_repo" not in sys.path:
    sys.path.insert(0, "/opt/trn_rl_repo")

import numpy as np

import concourse.bacc as bacc
import concourse.mybir as mybir
import concourse.tile as tile
from concourse.bass_utils import run_bass_kernel_spmd
from concourse.masks import make_identity

F32 = mybir.dt.float32
F16 = mybir.dt.float16
NPDT = np.float16

B, S, D = 2, 2048, 1024
NH, DH = 16, 64
NCORES = 8
GROUPS = 4                # head-groups (cores per batch)
HG = NH // GROUPS         # heads per core = 4
IS = HG * DH              # inner slice per core = 256
KD = D // 128             # contraction chunks for projections = 8
MT = IS // 128            # m-tiles of the inner slice = 2
W = 1024                  # seq slab width (fp16 moving-operand max)
NSLAB = S // W            # seq slabs = 2
KT = S // 128             # 128-row key chunks = 16
QT = S // 128             # 128-row query tiles = 16

_CACHE = {}


def _build_nc():
    nc = bacc.Bacc("TRN2", target_bir_lowering=False, debug=False)

    xqT = nc.dram_tensor("xqT", [D, S], F16, kind="ExternalInput").ap()
    xkT = nc.dram_tensor("xkT", [D, S], F16, kind="ExternalInput").ap()
    xvT = nc.dram_tensor("xvT", [D, S], F16, kind="ExternalInput").ap()
    wq = nc.dram_tensor("wq", [D, IS], F16, kind="ExternalInput").ap()
    wk = nc.dram_tensor("wk", [D, IS], F16, kind="ExternalInput").ap()
    wv = nc.dram_tensor("wv", [D, IS], F16, kind="ExternalInput").ap()
    wo = nc.dram_tensor("wo", [IS, D], F16, kind="ExternalInput").ap()
    bq = nc.dram_tensor("bq", [IS], F32, kind="ExternalInput").ap()
    bk = nc.dram_tensor("bk", [IS], F32, kind="ExternalInput").ap()
    bv = nc.dram_tensor("bv", [IS], F32, kind="ExternalInput").ap()
    out = nc.dram_tensor("out", [S, D], F32, kind="ExternalOutput").ap()

    with tile.TileContext(nc) as tc:
        _emit(nc, tc, xqT, xkT, xvT, wq, wk, wv, wo, bq, bk, bv, out)
    nc.compile()
    return nc


def _emit(nc, tc, xqT, xkT, xvT, wq, wk, wv, wo, bq, bk, bv, out):
    from contextlib import ExitStack

    def mm(out_ap, lhsT, rhs, start, stop):
        """Matmul with the fp16 moving-operand N<=512 limit: slice the
        free dim of out/rhs into 512-wide chunks."""
        n = rhs.shape[-1]
        assert out_ap.shape[-1] == n
        for o in range(0, n, 512):
            w = min(512, n - o)
            nc.tensor.matmul(
                out_ap[..., o:o + w], lhsT, rhs[..., o:o + w],
                start=start, stop=stop,
            )

    ctx = ExitStack()
    with ctx:
        consts = ctx.enter_context(tc.tile_pool(name="consts", bufs=1))
        big = ctx.enter_context(tc.tile_pool(name="big", bufs=1))
        xin = ctx.enter_context(tc.tile_pool(name="xin", bufs=6))
        expp = ctx.enter_context(tc.tile_pool(name="expp", bufs=8))
        smallp = ctx.enter_context(tc.tile_pool(name="smallp", bufs=2))
        outp = ctx.enter_context(tc.tile_pool(name="outp", bufs=3))

        # ---- constants (weight/bias DMAs are emitted per-stage below so
        # the first projection's inputs go out on the wire first) ----
        ident = consts.tile([128, 128], F16, name="ident")
        make_identity(nc, ident)
        col1_f = consts.tile([128, 1], F16, name="col1_f")
        nc.vector.memset(col1_f, 1.0)
        wq_sb = consts.tile([128, KD, IS], F16, name="wq_sb")
        wk_sb = consts.tile([128, KD, IS], F16, name="wk_sb")
        wv_sb = consts.tile([128, KD, IS], F16, name="wv_sb")
        wo_sb = consts.tile([128, MT, D], F16, name="wo_sb")
        bq_sb = consts.tile([128, MT], F32, name="bq_sb")
        bk_sb = consts.tile([128, MT], F32, name="bk_sb")
        bv_sb = consts.tile([128, MT], F32, name="bv_sb")

        # ---- persistent intermediates ----
        # Q^T/K^T are stored per-head, zero-padded from dh=64 to a full
        # 128-row contraction, and V chunks are zero-padded from 65 to
        # 128 columns: a K=64 / M=65 matmul only lights half the PE
        # array's row/col groups and the HAM clock gate then never sees
        # "busy", locking the PE at 1.2 GHz through all of attention.
        # The padding costs nothing (matmul time is N cycles regardless
        # of K and M) and keeps the array at 2.4 GHz.
        QT_sb = big.tile([128, HG, S], F16, name="QT_sb")
        KT_sb = big.tile([128, HG, S], F16, name="KT_sb")
        VT_sb = big.tile([128, MT, S], F16, name="VT_sb")
        V_sb = big.tile([128, HG * KT, 128], F16, name="V_sb")
        OT_sb = big.tile([128, MT, S], F16, name="OT_sb")
        nc.gpsimd.memset(QT_sb, 0.0)
        nc.gpsimd.memset(KT_sb, 0.0)
        nc.gpsimd.memset(V_sb, 0.0)
        # the "ones" column (col DH) of every V chunk
        nc.vector.tensor_copy(
            V_sb[:, :, DH:DH + 1],
            col1_f.unsqueeze(1).broadcast_to([128, HG * KT, 1]),
        )

        # PSUM->SBUF moves with bias add, alternating between DVE and the
        # (otherwise idle in stage 1) ACT engine
        _eng = [0]

        def bias_copy(dst_ap, src_ap, bias_ap):
            if _eng[0] % 2 == 0:
                nc.vector.tensor_scalar_add(dst_ap, src_ap, bias_ap)
            else:
                nc.scalar.activation(
                    dst_ap, src_ap,
                    mybir.ActivationFunctionType.Identity,
                    bias=bias_ap,
                )
            _eng[0] += 1

        # ---- stage 1: projections into transposed layout [inner, seq] ----
        NS2 = S // 512
        with nc.named_scope("proj"):
            with tc.tile_pool(name="psP", bufs=8, space="PSUM") as psP:
                first = True
                for xT, w_dram, w_sb, b_dram, b_sb, dest in (
                    (xkT, wk, wk_sb, bk, bk_sb, KT_sb),
                    (xqT, wq, wq_sb, bq, bq_sb, QT_sb),
                    (xvT, wv, wv_sb, bv, bv_sb, VT_sb),
                ):
                    nc.sync.dma_start(
                        out=w_sb, in_=w_dram.rearrange("(k p) i -> p k i", p=128)
                    )
                    nc.sync.dma_start(
                        out=b_sb, in_=b_dram.rearrange("(m p) -> p m", p=128)
                    )
                    ps = [
                        [
                            psP.tile([128, 512], F32, tag="ps", name=f"ps_{m}_{n}")
                            for n in range(NS2)
                        ]
                        for m in range(MT)
                    ]
                    for k in range(KD):
                        xt = xin.tile([128, S], F16, tag="xt")
                        nc.sync.dma_start(out=xt, in_=xT[128 * k:128 * (k + 1), :])
                        if first and k == 0:
                            # HAM warm-up: burn ~7us of PE on a throwaway
                            # pass over chunk 0 while the remaining input
                            # DMAs stream in, so the clock gate opens
                            # before the real work.
                            for m in range(MT):
                                for n in range(NS2):
                                    nc.tensor.matmul(
                                        ps[m][n],
                                        w_sb[:, 0, 128 * m:128 * (m + 1)],
                                        xt[:, 512 * n:512 * (n + 1)],
                                        start=True, stop=True)
                            first = False
                        for m in range(MT):
                            for n in range(NS2):
                                nc.tensor.matmul(
                                    ps[m][n],
                                    w_sb[:, k, 128 * m:128 * (m + 1)],
                                    xt[:, 512 * n:512 * (n + 1)],
                                    start=(k == 0),
                                    stop=(k == KD - 1),
                                )
                    for m in range(MT):
                        for n in range(NS2):
                            if dest is VT_sb:
                                bias_copy(
                                    dest[:, m, 512 * n:512 * (n + 1)],
                                    ps[m][n],
                                    b_sb[:, m:m + 1],
                                )
                            else:
                                for hh in range(2):
                                    bias_copy(
                                        dest[0:DH, 2 * m + hh, 512 * n:512 * (n + 1)],
                                        ps[m][n][64 * hh:64 * hh + DH, :],
                                        b_sb[64 * hh:64 * hh + DH, m:m + 1],
                                    )
                nc.sync.dma_start(
                    out=wo_sb, in_=wo.rearrange("(c p) d -> p c d", p=128)
                )

        # ---- stages 1.5 + 2 + 3 share one PSUM pool: tag "sc" (3x
        # 2-bank slots, reused by V-transposes, score tiles, and
        # output-projection tiles) + tag "av" (2x 1-bank halves) ----
        with tc.tile_pool(name="psX", bufs=2, space="PSUM") as psX:
            # ---- stage 1.5: V^T -> V natural chunks (PE transpose).
            # Only the m=0 tile (heads 0/1) is emitted up front; the m=1
            # transposes are drip-fed into the early attention stream as
            # PE filler (heads 2/3 don't need them until ~30 chunks in),
            # which also keeps the HAM clock gate warm through the
            # stage boundary. ----
            def emit_vtr(m, j):
                trp = psX.tile([128, 128], F16, tag="sc", name="trp",
                               bufs=3)
                nc.tensor.transpose(
                    trp, VT_sb[:, m, 128 * j:128 * (j + 1)], ident
                )
                for hh in range(2):
                    h = 2 * m + hh
                    nc.vector.tensor_copy(
                        V_sb[:, h * KT + j, 0:DH],
                        trp[:, 64 * hh:64 * hh + DH],
                    )

            vtr_queue = [(1, j) for j in range(KT)]
            with nc.named_scope("vtr"):
                for j in range(KT):
                    emit_vtr(0, j)

            # ---- stages 2 + 3: one flat software-pipelined stream over
            # (head, k-chunk): attnV for chunk i-2 is emitted after
            # scores+exp for chunk i (the PE then never waits on the ACT
            # exp, which is the phase bottleneck, and a stalled PE
            # sequencer would re-throttle the clock), crossing head
            # boundaries so the ACT pipe never drains. Output-projection
            # tiles for the previous q-slab drip in as PE filler. ----
            obs = {}
            _foeng = [0]

            def emit_fo(t, half, tag="sc"):
                fp = psX.tile([128, 512], F32, tag=tag, name="fp",
                              bufs=3 if tag == "sc" else 2)
                for c in range(MT):
                    nc.tensor.matmul(
                        fp,
                        OT_sb[:, c, 128 * t:128 * (t + 1)],
                        wo_sb[:, c, 512 * half:512 * (half + 1)],
                        start=(c == 0),
                        stop=(c == MT - 1),
                    )
                if t not in obs:
                    obs[t] = outp.tile([128, D], F32, tag="ob", name="ob")
                dst = obs[t][:, 512 * half:512 * (half + 1)]
                if _foeng[0] % 2 == 0:
                    nc.vector.tensor_copy(dst, fp)
                else:
                    nc.scalar.activation(
                        dst, fp, mybir.ActivationFunctionType.Copy
                    )
                _foeng[0] += 1
                if half == 1:
                    nc.sync.dma_start(
                        out=out[128 * t:128 * (t + 1), :], in_=obs.pop(t)
                    )

            fo_queue = []
            with nc.named_scope("attn"):
                avs = {}
                exs = {}

                def attnv(s, h, j):
                    if j == 0:
                        avs[(s, h)] = [
                            psX.tile([128, 512], F32, tag="av",
                                     name=f"av{half}", bufs=2)
                            for half in range(2)
                        ]
                    ex = exs.pop((s, h, j))
                    for half in range(2):
                        nc.tensor.matmul(
                            avs[(s, h)][half],
                            V_sb[:, h * KT + j, :],
                            ex[:, 512 * half:512 * (half + 1)],
                            start=(j == 0),
                            stop=(j == KT - 1),
                        )

                def norm(s, h):
                    m_h, p0 = h // 2, 64 * (h % 2)
                    av = avs.pop((s, h))
                    sums = smallp.tile([1, W], F32, tag="sums")
                    for half in range(2):
                        nc.vector.tensor_copy(
                            sums[:, 512 * half:512 * (half + 1)],
                            av[half][DH:DH + 1, :],
                        )
                    rec = smallp.tile([1, W], F32, tag="rec")
                    nc.vector.reciprocal_approx_fast(rec, sums)
                    bcs = smallp.tile([DH, W], F32, tag="bcs")
                    nc.gpsimd.partition_broadcast(bcs, rec)
                    for half in range(2):
                        nc.vector.tensor_mul(
                            OT_sb[p0:p0 + DH, m_h,
                                  W * s + 512 * half:W * s + 512 * (half + 1)],
                            av[half][0:DH, :],
                            bcs[:, 512 * half:512 * (half + 1)],
                        )
                    if h == HG - 1:
                        # slab finished: queue its output projection as
                        # PE filler for the rest of the stream
                        fo_queue.extend(
                            (t, half)
                            for t in range(W * s // 128, W * (s + 1) // 128)
                            for half in range(2)
                        )

                NIT = NSLAB * HG * KT
                for idx in range(NIT + 2):
                    if idx < NIT:
                        s, hj = divmod(idx, HG * KT)
                        h, j = divmod(hj, KT)
                        sc = psX.tile([128, W], F32, tag="sc", name="sc",
                                      bufs=3)
                        mm(
                            sc,
                            KT_sb[:, h, 128 * j:128 * (j + 1)],
                            QT_sb[:, h, W * s:W * (s + 1)],
                            start=True,
                            stop=True,
                        )
                        ex = expp.tile([128, W], F16, tag="ex")
                        nc.scalar.activation(
                            ex, sc, mybir.ActivationFunctionType.Exp,
                            scale=0.125,
                        )
                        exs[(s, h, j)] = ex
                    if idx >= 2:
                        sl, hjl = divmod(idx - 2, HG * KT)
                        hl, jl = divmod(hjl, KT)
                        attnv(sl, hl, jl)
                        if jl == KT - 1:
                            norm(sl, hl)
                    if idx % 2 == 1 and vtr_queue:
                        emit_vtr(*vtr_queue.pop(0))
                    elif idx % 4 == 3 and fo_queue:
                        emit_fo(*fo_queue.pop(0))
            # drain the last slab's output projection, alternating PSUM
            # tags for a deeper pipeline
            with nc.named_scope("outproj"):
                for i, item in enumerate(fo_queue):
                    emit_fo(*item, tag=("sc", "av")[i % 2])


def _get_nc():
    if "nc" not in _CACHE:
        _CACHE["nc"] = _build_nc()
    return _CACHE["nc"]


def make_in_maps(query, key, value, Wq, bq, Wk, bk, Wv, bv, Wo, bo):
    f32 = lambda a: np.asarray(a, dtype=np.float32)
    f16 = lambda a: np.ascontiguousarray(np.asarray(a, dtype=np.float32).astype(NPDT))
    query, key, value = f32(query), f32(key), f32(value)
    bq, bk, bv = (
        np.ascontiguousarray(f32(bq)),
        np.ascontiguousarray(f32(bk)),
        np.ascontiguousarray(f32(bv)),
    )
    Wq, Wk, Wv, Wo = f32(Wq), f32(Wk), f32(Wv), f32(Wo)

    xT = [[f16(x[b].T) for b in range(B)] for x in (query, key, value)]
    in_maps = []
    for c in range(NCORES):
        b, g = c // GROUPS, c % GROUPS
        sl = slice(IS * g, IS * (g + 1))
        in_maps.append({
            "xqT": xT[0][b],
            "xkT": xT[1][b],
            "xvT": xT[2][b],
            "wq": f16(Wq[:, sl]),
            "wk": f16(Wk[:, sl]),
            "wv": f16(Wv[:, sl]),
            "wo": f16(Wo[sl, :]),
            "bq": np.ascontiguousarray(bq[sl]),
            "bk": np.ascontiguousarray(bk[sl]),
            "bv": np.ascontiguousarray(bv[sl]),
        })
    return in_maps


def combine_outputs(results, bo):
    bo = np.asarray(bo, dtype=np.float32)
    out = np.empty((B, S, D), dtype=np.float32)
    for b in range(B):
        acc = results[b * GROUPS]["out"].astype(np.float32)
        for g in range(1, GROUPS):
            acc = acc + results[b * GROUPS + g]["out"]
        out[b] = acc + bo
    return out


def kernel(query, key, value, Wq, bq, Wk, bk, Wv, bv, Wo, bo):
    nc = _get_nc()
    in_maps = make_in_maps(query, key, value, Wq, bq, Wk, bk, Wv, bv, Wo, bo)
    try:
        res = run_bass_kernel_spmd(nc, in_maps, list(range(NCORES)))
    except Exception:
        # a fresh NEFF's first execution occasionally reports
        # NRT_EXEC_UNIT_UNRECOVERABLE; a retry reliably succeeds
        res = run_bass_kernel_spmd(nc, in_maps, list(range(NCORES)))
    return combine_outputs(res.results, bo)
